# revision 10
# baseline (speedup 1.0000x reference)
"""BEV view transform (lift-splat segment-sum) Trainium2 kernel.

Strategy
--------
Host: compute per-point BEV cell ids from the small calibration matrices
(exactly mirroring the reference jax ops), then shard points across the 8
NeuronCores by cell % 8 (output-interleaved, perfectly balanced) and build a
cell-sorted, chunk-padded compact copy of the kept feature rows per core.

Device (one SPMD program on 8 cores): stream the compact rows sequentially,
build one-hot matrices from per-point local-cell metadata (DVE is_equal vs an
iota row), accumulate each "group" (a 128-wide window of the core's cell
columns) in PSUM via chained matmuls (onehot^T @ feats -> [128 cells, 80]),
then flush each window into a per-core DRAM table [SPAN, 128] with
dma_scatter_add (row = local cell), tails redirected to a dump window.
Host reassembles [1, 80, 360, 360] from the 8 tables.

FLUSH_MODE:
  "scatter" (default): dma_scatter_add flush, no registers.
  "reg": SBUF accumulator + dynamic-offset DVE adds (smaller DMA traffic).
"""
import os
import numpy as np

# ---- problem constants (hardcoded; must match the reference task) ----
IMAGE_SIZE = (256, 704)
FEATURE_SIZE = (32, 88)
XBOUND = (-54.0, 54.0, 0.3)
YBOUND = (-54.0, 54.0, 0.3)
ZBOUND = (-10.0, 10.0, 20.0)
DBOUND = (1.0, 60.0, 0.5)
NXX = 360
NXY = 360
NXZ = 1
C = 80
P = 128

N_CORES = 8
SHARD_COLS = (NXX * NXY) // N_CORES          # 16200 local columns per core
WIN = 128                                    # group window (accumulator cols)
SPAN = 16384                                 # table rows / accumulator width
DUMP_BASE = SPAN - WIN                       # scatter dump window
GROUP_PT_CAP = 2048                          # max points per group (16 chunks)
MAX_CLS = GROUP_PT_CAP // P                  # 16
J_TILE = 32                                  # chunks per feats DMA tile
G_BATCH = 16                                 # groups per scatter_add call
PAD_LCELL = 20000.0                          # local-cell value for padding lanes
FLUSH_MODE = os.environ.get("BEV_FLUSH", "scatter")
ONEHOT_BATCH3D = os.environ.get("BEV_ONEHOT_3D", "1") == "1"
DTYPE = os.environ.get("BEV_DTYPE", "hilo")  # "f32" | "bf16" | "hilo"

LAST_RESULTS = None  # test harness can inspect exec_time_ns / profile


def _geometry_coords(camera2lidar, camera_intrinsics, img_aug_matrix, lidar_aug_matrix):
    """Per-point voxel coords, bit-identical to the reference jax computation.

    Runs on CPU jax: the reference graph itself only runs on CPU (neuron has
    no triangular-solve for linalg.inv), so the grading reference output is
    CPU-computed and we must match its float behavior exactly.
    """
    import jax
    with jax.default_device(jax.devices('cpu')[0]):
        return _geometry_coords_impl(camera2lidar, camera_intrinsics,
                                     img_aug_matrix, lidar_aug_matrix)


def _geometry_coords_impl(camera2lidar, camera_intrinsics, img_aug_matrix, lidar_aug_matrix):
    import jax.numpy as jnp
    from jax import lax

    DX = np.array([XBOUND[2], YBOUND[2], ZBOUND[2]], np.float32)
    BX = np.array([XBOUND[0] + XBOUND[2] / 2.0,
                   YBOUND[0] + YBOUND[2] / 2.0,
                   ZBOUND[0] + ZBOUND[2] / 2.0], np.float32)

    iH, iW = IMAGE_SIZE
    fH, fW = FEATURE_SIZE
    ds = jnp.arange(DBOUND[0], DBOUND[1], DBOUND[2], dtype=jnp.float32)
    xs = jnp.linspace(0.0, iW - 1.0, fW, dtype=jnp.float32)
    ys = jnp.linspace(0.0, iH - 1.0, fH, dtype=jnp.float32)
    d_g, y_g, x_g = jnp.meshgrid(ds, ys, xs, indexing='ij')
    frustum = jnp.stack([x_g, y_g, d_g], axis=-1)            # [D,fH,fW,3]

    camera2lidar = jnp.asarray(camera2lidar)
    camera_intrinsics = jnp.asarray(camera_intrinsics)
    img_aug_matrix = jnp.asarray(img_aug_matrix)
    lidar_aug_matrix = jnp.asarray(lidar_aug_matrix)

    intrins = camera_intrinsics[..., :3, :3]
    post_rots = img_aug_matrix[..., :3, :3]
    post_trans = img_aug_matrix[..., :3, 3]
    c2l_rots = camera2lidar[..., :3, :3]
    c2l_trans = camera2lidar[..., :3, 3]
    extra_rots = lidar_aug_matrix[..., :3, :3]
    extra_trans = lidar_aug_matrix[..., :3, 3]

    pts = frustum[None, None] - post_trans[:, :, None, None, None, :]
    pts = jnp.einsum('bnij,bndhwj->bndhwi', jnp.linalg.inv(post_rots), pts)
    pts = jnp.concatenate([pts[..., :2] * pts[..., 2:3], pts[..., 2:3]], axis=-1)
    combine = c2l_rots @ jnp.linalg.inv(intrins)
    pts = jnp.einsum('bnij,bndhwj->bndhwi', combine, pts) + c2l_trans[:, :, None, None, None, :]
    pts = jnp.einsum('bij,bndhwj->bndhwi', extra_rots, pts) + extra_trans[:, None, None, None, None, :]

    dx = jnp.asarray(DX)
    bx = jnp.asarray(BX)
    coords = ((lax.stop_gradient(pts) - (bx - dx / 2.0)) / dx).astype(jnp.int32)
    return np.asarray(coords).reshape(-1, 3)


def _build_metadata(coords):
    """Shard + sort + group.  Returns per-core metadata and structure info."""
    xi = coords[:, 0].astype(np.int64)
    yi = coords[:, 1].astype(np.int64)
    zi = coords[:, 2].astype(np.int64)
    kept = (xi >= 0) & (xi < NXX) & (yi >= 0) & (yi < NXY) & (zi >= 0) & (zi < NXZ)
    cell = xi * NXY + yi
    kept_rows = np.nonzero(kept)[0].astype(np.int64)
    cellk = cell[kept_rows]
    order = np.argsort(cellk, kind='stable')
    cellk = cellk[order]
    rowsk = kept_rows[order]

    cores = []
    for c in range(N_CORES):
        sel = (cellk % N_CORES) == c
        cores.append((cellk[sel] // N_CORES, rowsk[sel]))  # (local col sorted asc, src row)

    # group formation per core: window WIN local cols, <= GROUP_PT_CAP points,
    # whole cells only; class = exact number of chunks.
    per_core_groups = []
    for c in range(N_CORES):
        cols, rows = cores[c]
        cu, cstart = np.unique(cols, return_index=True)
        ccount = np.diff(np.append(cstart, len(cols)))
        groups = []
        i, n = 0, len(cu)
        while i < n:
            base = cu[i]
            pts = 0
            j = i
            while j < n and cu[j] < base + WIN and pts + ccount[j] <= GROUP_PT_CAP:
                pts += ccount[j]
                j += 1
            width = int(cu[j - 1] - base + 1)
            groups.append((int(base), i, j, int(pts), width))
            i = j
        per_core_groups.append((groups, cu, cstart, ccount, rows))

    # class counts -> per-class max across cores
    cls_counts = np.zeros((N_CORES, MAX_CLS + 1), np.int64)
    for c in range(N_CORES):
        for (base, i0, i1, pts, width) in per_core_groups[c][0]:
            cls_counts[c, (pts + P - 1) // P] += 1
    cls_max = cls_counts.max(axis=0)
    n_groups_tot = int(cls_max[1:].sum())
    # round groups to a multiple of G_BATCH (extra dummy groups, class 1)
    ng_round = ((n_groups_tot + G_BATCH - 1) // G_BATCH) * G_BATCH
    cls_max[1] += ng_round - n_groups_tot
    n_groups_tot = ng_round
    nch = int(sum(cls * cls_max[cls] for cls in range(1, MAX_CLS + 1)))
    nch = ((nch + J_TILE - 1) // J_TILE) * J_TILE

    structure = []                            # chunk-count per group, program order
    for cls in range(MAX_CLS, 0, -1):
        structure.extend([cls] * int(cls_max[cls]))

    per_core_meta = []
    for c in range(N_CORES):
        groups, cu, cstart, ccount, rows = per_core_groups[c]
        by_cls = {cls: [] for cls in range(1, MAX_CLS + 1)}
        for g in groups:
            by_cls[(g[3] + P - 1) // P].append(g)

        src = np.full(nch * P, -1, np.int64)      # source row in x (or -1 pad)
        lcell = np.full((P, nch), PAD_LCELL, np.float32)
        bases = np.zeros(n_groups_tot, np.int32)
        widths = np.zeros(n_groups_tot, np.int32)

        chunk_cursor = 0
        gslot = 0
        for cls in range(MAX_CLS, 0, -1):
            glist = by_cls[cls]
            for k in range(int(cls_max[cls])):
                if k < len(glist):
                    base, i0, i1, pts, width = glist[k]
                    bases[gslot] = base
                    widths[gslot] = width
                    s0 = cstart[i0]
                    seg_rows = rows[s0:s0 + pts]
                    seg_cols = (cu[np.repeat(np.arange(i0, i1), ccount[i0:i1])] - base)
                    o = chunk_cursor * P
                    src[o:o + pts] = seg_rows
                    ppos = np.arange(pts)
                    lcell[ppos % P, chunk_cursor + ppos // P] = seg_cols.astype(np.float32)
                else:
                    bases[gslot] = 0
                    widths[gslot] = 0
                gslot += 1
                chunk_cursor += cls
        # int16 scatter indices per group: [128] = base+t (t<width) else dump
        t = np.arange(WIN)
        sidx = np.where(t[None, :] < widths[:, None],
                        bases[:, None] + t[None, :],
                        DUMP_BASE + t[None, :]).astype(np.int16)  # [NG, 128]
        # wrap: ordinal i -> partition i%16, slot i//16; replicate across 8 Q7 groups
        NG = n_groups_tot
        wrapped = np.zeros((NG, 16, WIN // 16), np.int16)
        ii = np.arange(WIN)
        wrapped[:, ii % 16, ii // 16] = sidx[:, ii]
        # [NG, 128, W16] replicated across Q7 groups -> SBUF layout [128, NG*W16]
        sidx_w = np.ascontiguousarray(
            np.tile(wrapped, (1, 8, 1)).transpose(1, 0, 2)).reshape(P, -1)
        per_core_meta.append((src, lcell, bases, sidx_w))

    return per_core_meta, structure, nch, n_groups_tot


def _np_dtype():
    if DTYPE in ("bf16", "hilo"):
        import ml_dtypes
        return ml_dtypes.bfloat16
    return np.float32


def _row_width():
    return 2 * C if DTYPE == "hilo" else C


def _compact_rows(x_rows, src, nch):
    """Gather kept rows (cell-sorted, chunk-padded) into the DMA-tile layout.

    Device reads tiles of 4096 rows into SBUF [128, J*80]: compact row
    (t*4096 + p*J + j) must hold stream position (t*J + j)*128 + p.
    """
    npts = nch * P
    cw = _row_width()
    if DTYPE == "hilo":
        stream32 = np.zeros((npts, C), np.float32)
        valid = src >= 0
        stream32[valid] = x_rows[src[valid]]
        hi = stream32.astype(_np_dtype())
        lo = (stream32 - hi.astype(np.float32)).astype(_np_dtype())
        stream = np.concatenate([hi, lo], axis=1)          # [npts, 160]
    else:
        stream = np.zeros((npts, cw), _np_dtype())
        valid = src >= 0
        stream[valid] = x_rows[src[valid]]
    ntiles = nch // J_TILE
    return np.ascontiguousarray(
        stream.reshape(ntiles, J_TILE, P, cw).transpose(0, 2, 1, 3)
    ).reshape(npts, cw)


def _build_program(nch, n_groups_tot, structure):
    import concourse.mybir as mybir
    import concourse.tile as tile
    import concourse.bass as bass
    from concourse import bacc
    from concourse.bass import AP

    nc = bacc.Bacc(None)
    fdt = mybir.dt.bfloat16 if DTYPE in ("bf16", "hilo") else mybir.dt.float32
    cw = 2 * C if DTYPE == "hilo" else C
    xc_d = nc.declare_dram_parameter("xc", [nch * P, cw], fdt, isOutput=False)
    lcell_d = nc.declare_dram_parameter("lcell", [P, nch], fdt, isOutput=False)
    scatter_mode = FLUSH_MODE == "scatter"
    if scatter_mode:
        sidx_d = nc.declare_dram_parameter(
            "sidx", [P, n_groups_tot * (WIN // 16)], mybir.dt.int16, isOutput=False)
        out_d = nc.declare_dram_parameter("out", [SPAN, P], mybir.dt.float32, isOutput=True)
    else:
        bases_d = nc.declare_dram_parameter("bases", [1, n_groups_tot], mybir.dt.int32, isOutput=False)
        out_d = nc.declare_dram_parameter("out", [C, SPAN], mybir.dt.float32, isOutput=True)

    with tile.TileContext(nc) as tc:
        with (
            tc.tile_pool(name="meta", bufs=1) as meta,
            tc.tile_pool(name="feats", bufs=4) as featsp,
            tc.tile_pool(name="oneh", bufs=4) as onehp,
            tc.tile_pool(name="psum", bufs=4, space="PSUM") as psump,
            tc.tile_pool(name="stage", bufs=3) as stagep,
            tc.tile_pool(name="const", bufs=1) as constp,
        ):
            lcell_sb = meta.tile([P, nch], fdt)
            nc.sync.dma_start(lcell_sb[:], lcell_d[:])
            if scatter_mode:
                sidx_sb = meta.tile([P, n_groups_tot * (WIN // 16)], mybir.dt.int16)
                nc.sync.dma_start(sidx_sb[:], sidx_d[:])
            else:
                bases_sb = meta.tile([1, n_groups_tot], mybir.dt.int32)
                nc.sync.dma_start(bases_sb[:], bases_d[:])
                accum = stagep.tile([C, SPAN], mybir.dt.float32, tag="acc")
                nc.vector.memset(accum[:], 0.0)

            iota_i = constp.tile([P, WIN], mybir.dt.int32)
            nc.gpsimd.iota(iota_i[:], pattern=[[1, WIN]], base=0, channel_multiplier=0)
            iota_f = constp.tile([P, WIN], fdt)
            nc.vector.tensor_copy(iota_f[:], iota_i[:])

            feats_tiles = [None] * (nch // J_TILE)
            oneh_tiles = [None] * (nch // J_TILE)

            def ensure_tile(t):
                if feats_tiles[t] is None:
                    ft = featsp.tile([P, J_TILE * cw], fdt)
                    nc.sync.dma_start(
                        ft[:], xc_d[t * J_TILE * P:(t + 1) * J_TILE * P, :])
                    feats_tiles[t] = ft
                    oh = onehp.tile([P, J_TILE * WIN], fdt)
                    if ONEHOT_BATCH3D:
                        lc = lcell_sb[:, t * J_TILE:(t + 1) * J_TILE]
                        in0 = AP(lc.tensor, lc.offset, list(lc.ap) + [[0, WIN]])
                        it = iota_f[:]
                        in1 = AP(it.tensor, it.offset,
                                 [list(it.ap[0]), [0, J_TILE], list(it.ap[1])])
                        out_oh = oh[:].rearrange("p (j w) -> p j w", w=WIN)
                        nc.vector.tensor_tensor(out=out_oh, in0=in0, in1=in1,
                                                op=mybir.AluOpType.is_equal)
                    else:
                        for s in range(J_TILE):
                            nc.vector.tensor_tensor(
                                out=oh[:, s * WIN:(s + 1) * WIN],
                                in0=lcell_sb[:, t * J_TILE + s:t * J_TILE + s + 1].to_broadcast([P, WIN]),
                                in1=iota_f[:],
                                op=mybir.AluOpType.is_equal)
                    oneh_tiles[t] = oh

            cursor = 0
            stage_tile = None
            for g, cls in enumerate(structure):
                if scatter_mode:
                    ps = psump.tile([WIN, C], mybir.dt.float32)
                else:
                    ps = psump.tile([C, WIN], mybir.dt.float32)
                for t in range(cls):
                    j = cursor + t
                    tid, slot = j // J_TILE, j % J_TILE
                    ensure_tile(tid)
                    oneh_ap = oneh_tiles[tid][:, slot * WIN:(slot + 1) * WIN]
                    if DTYPE == "hilo":
                        hi_ap = feats_tiles[tid][:, slot * cw:slot * cw + C]
                        lo_ap = feats_tiles[tid][:, slot * cw + C:(slot + 1) * cw]
                        nc.tensor.matmul(out=ps[:], lhsT=oneh_ap, rhs=hi_ap,
                                         start=(t == 0), stop=False)
                        nc.tensor.matmul(out=ps[:], lhsT=oneh_ap, rhs=lo_ap,
                                         start=False, stop=(t == cls - 1))
                        cursorless = None
                    else:
                        feats_ap = feats_tiles[tid][:, slot * C:(slot + 1) * C]
                        if scatter_mode:
                            nc.tensor.matmul(out=ps[:], lhsT=oneh_ap, rhs=feats_ap,
                                             start=(t == 0), stop=(t == cls - 1))
                        else:
                            nc.tensor.matmul(out=ps[:], lhsT=feats_ap, rhs=oneh_ap,
                                             start=(t == 0), stop=(t == cls - 1))
                cursor += cls
                if scatter_mode:
                    gb = g % G_BATCH
                    if gb == 0:
                        stage_tile = stagep.tile([P, G_BATCH * C], mybir.dt.float32)
                    nc.vector.tensor_copy(stage_tile[:, gb * C:(gb + 1) * C], ps[:])
                    if gb == G_BATCH - 1:
                        g0 = g - (G_BATCH - 1)
                        W16 = WIN // 16
                        nc.gpsimd.dma_scatter_add(
                            out_d[:, :C],
                            stage_tile[:].rearrange("p (b e) -> p b e", e=C),
                            sidx_sb[:, g0 * W16:(g0 + G_BATCH) * W16],
                            G_BATCH * WIN, G_BATCH * WIN, C,
                            elem_step=P)
                else:
                    rv = nc.vector.value_load(bases_sb[0:1, g:g + 1],
                                              min_val=0, max_val=SPAN - WIN)
                    dst = accum[:, bass.ds(rv, WIN)]
                    nc.vector.tensor_tensor(out=dst, in0=dst, in1=ps[:],
                                            op=mybir.AluOpType.add)

            if not scatter_mode:
                nc.sync.dma_start(out_d[:], accum[:])
    nc.finalize()
    return nc


def kernel(x, camera2lidar, camera_intrinsics, img_aug_matrix, lidar_aug_matrix):
    global LAST_RESULTS
    from concourse.bass_utils import run_bass_kernel_spmd

    x = np.ascontiguousarray(np.asarray(x, dtype=np.float32))
    B, N, D, H, W, Cin = x.shape
    assert (B, N, D, H, W, Cin) == (1, 6, 118, 32, 88, 80), x.shape

    coords = _geometry_coords(camera2lidar, camera_intrinsics,
                              img_aug_matrix, lidar_aug_matrix)
    per_core_meta, structure, nch, n_groups_tot = _build_metadata(coords)

    x_rows = x.reshape(-1, C)
    if DTYPE == "bf16":
        x_rows = x_rows.astype(_np_dtype())
    scatter_mode = FLUSH_MODE == "scatter"
    in_maps = []
    for c in range(N_CORES):
        src, lcell, bases, sidx_w = per_core_meta[c]
        m = {"xc": _compact_rows(x_rows, src, nch),
             "lcell": lcell.astype(_np_dtype())}
        if scatter_mode:
            m["sidx"] = sidx_w
        else:
            m["bases"] = bases.reshape(1, -1)
        in_maps.append(m)

    nc = _build_program(nch, n_groups_tot, structure)
    res = run_bass_kernel_spmd(nc, in_maps, list(range(N_CORES)))
    LAST_RESULTS = res

    out = np.zeros((C, NXX * NXY), np.float32)
    for c in range(N_CORES):
        if scatter_mode:
            shard = res.results[c]["out"][:SHARD_COLS, :C].T
        else:
            shard = res.results[c]["out"][:, :SHARD_COLS]
        out[:, c::N_CORES] = shard
    return out.reshape(1, C, NXX, NXY)


# revision 11
# speedup vs baseline: 1.0827x; 1.0827x over previous
"""BEV view transform (lift-splat segment-sum) Trainium2 kernel.

Strategy
--------
Host: compute per-point BEV cell ids from the small calibration matrices
(exactly mirroring the reference jax ops), then shard points across the 8
NeuronCores by cell % 8 (output-interleaved, perfectly balanced) and build a
cell-sorted, chunk-padded compact copy of the kept feature rows per core.

Device (one SPMD program on 8 cores): stream the compact rows sequentially,
build one-hot matrices from per-point local-cell metadata (DVE is_equal vs an
iota row), accumulate each "group" (a 128-wide window of the core's cell
columns) in PSUM via chained matmuls (onehot^T @ feats -> [128 cells, 80]),
then flush each window into a per-core DRAM table [SPAN, 128] with
dma_scatter_add (row = local cell), tails redirected to a dump window.
Host reassembles [1, 80, 360, 360] from the 8 tables.

FLUSH_MODE:
  "scatter" (default): dma_scatter_add flush, no registers.
  "reg": SBUF accumulator + dynamic-offset DVE adds (smaller DMA traffic).
"""
import os
import numpy as np

# ---- problem constants (hardcoded; must match the reference task) ----
IMAGE_SIZE = (256, 704)
FEATURE_SIZE = (32, 88)
XBOUND = (-54.0, 54.0, 0.3)
YBOUND = (-54.0, 54.0, 0.3)
ZBOUND = (-10.0, 10.0, 20.0)
DBOUND = (1.0, 60.0, 0.5)
NXX = 360
NXY = 360
NXZ = 1
C = 80
P = 128

N_CORES = 8
SHARD_COLS = (NXX * NXY) // N_CORES          # 16200 local columns per core
WIN = 128                                    # group window (accumulator cols)
SPAN = 16384                                 # table rows / accumulator width
DUMP_BASE = SPAN - WIN                       # scatter dump window
GROUP_PT_CAP = 2048                          # max points per group (16 chunks)
MAX_CLS = GROUP_PT_CAP // P                  # 16
J_TILE = 32                                  # chunks per feats DMA tile
G_BATCH = 16                                 # groups per scatter_add call
PAD_LCELL = 20000.0                          # local-cell value for padding lanes
FLUSH_MODE = os.environ.get("BEV_FLUSH", "scatter")
ONEHOT_BATCH3D = os.environ.get("BEV_ONEHOT_3D", "1") == "1"
DTYPE = os.environ.get("BEV_DTYPE", "hilo")  # "f32" | "bf16" | "hilo"

LAST_RESULTS = None  # test harness can inspect exec_time_ns / profile


def _geometry_coords(camera2lidar, camera_intrinsics, img_aug_matrix, lidar_aug_matrix):
    """Per-point voxel coords, bit-identical to the reference jax computation.

    Runs on CPU jax: the reference graph itself only runs on CPU (neuron has
    no triangular-solve for linalg.inv), so the grading reference output is
    CPU-computed and we must match its float behavior exactly.
    """
    import jax
    with jax.default_device(jax.devices('cpu')[0]):
        return _geometry_coords_impl(camera2lidar, camera_intrinsics,
                                     img_aug_matrix, lidar_aug_matrix)


def _geometry_coords_impl(camera2lidar, camera_intrinsics, img_aug_matrix, lidar_aug_matrix):
    import jax.numpy as jnp
    from jax import lax

    DX = np.array([XBOUND[2], YBOUND[2], ZBOUND[2]], np.float32)
    BX = np.array([XBOUND[0] + XBOUND[2] / 2.0,
                   YBOUND[0] + YBOUND[2] / 2.0,
                   ZBOUND[0] + ZBOUND[2] / 2.0], np.float32)

    iH, iW = IMAGE_SIZE
    fH, fW = FEATURE_SIZE
    ds = jnp.arange(DBOUND[0], DBOUND[1], DBOUND[2], dtype=jnp.float32)
    xs = jnp.linspace(0.0, iW - 1.0, fW, dtype=jnp.float32)
    ys = jnp.linspace(0.0, iH - 1.0, fH, dtype=jnp.float32)
    d_g, y_g, x_g = jnp.meshgrid(ds, ys, xs, indexing='ij')
    frustum = jnp.stack([x_g, y_g, d_g], axis=-1)            # [D,fH,fW,3]

    camera2lidar = jnp.asarray(camera2lidar)
    camera_intrinsics = jnp.asarray(camera_intrinsics)
    img_aug_matrix = jnp.asarray(img_aug_matrix)
    lidar_aug_matrix = jnp.asarray(lidar_aug_matrix)

    intrins = camera_intrinsics[..., :3, :3]
    post_rots = img_aug_matrix[..., :3, :3]
    post_trans = img_aug_matrix[..., :3, 3]
    c2l_rots = camera2lidar[..., :3, :3]
    c2l_trans = camera2lidar[..., :3, 3]
    extra_rots = lidar_aug_matrix[..., :3, :3]
    extra_trans = lidar_aug_matrix[..., :3, 3]

    pts = frustum[None, None] - post_trans[:, :, None, None, None, :]
    pts = jnp.einsum('bnij,bndhwj->bndhwi', jnp.linalg.inv(post_rots), pts)
    pts = jnp.concatenate([pts[..., :2] * pts[..., 2:3], pts[..., 2:3]], axis=-1)
    combine = c2l_rots @ jnp.linalg.inv(intrins)
    pts = jnp.einsum('bnij,bndhwj->bndhwi', combine, pts) + c2l_trans[:, :, None, None, None, :]
    pts = jnp.einsum('bij,bndhwj->bndhwi', extra_rots, pts) + extra_trans[:, None, None, None, None, :]

    dx = jnp.asarray(DX)
    bx = jnp.asarray(BX)
    coords = ((lax.stop_gradient(pts) - (bx - dx / 2.0)) / dx).astype(jnp.int32)
    return np.asarray(coords).reshape(-1, 3)


def _build_metadata(coords):
    """Shard + sort + group.  Returns per-core metadata and structure info."""
    xi = coords[:, 0].astype(np.int64)
    yi = coords[:, 1].astype(np.int64)
    zi = coords[:, 2].astype(np.int64)
    kept = (xi >= 0) & (xi < NXX) & (yi >= 0) & (yi < NXY) & (zi >= 0) & (zi < NXZ)
    cell = xi * NXY + yi
    kept_rows = np.nonzero(kept)[0].astype(np.int64)
    cellk = cell[kept_rows]
    order = np.argsort(cellk, kind='stable')
    cellk = cellk[order]
    rowsk = kept_rows[order]

    cores = []
    for c in range(N_CORES):
        sel = (cellk % N_CORES) == c
        cores.append((cellk[sel] // N_CORES, rowsk[sel]))  # (local col sorted asc, src row)

    # group formation per core: window WIN local cols, <= GROUP_PT_CAP points,
    # whole cells only; class = exact number of chunks.
    per_core_groups = []
    for c in range(N_CORES):
        cols, rows = cores[c]
        cu, cstart = np.unique(cols, return_index=True)
        ccount = np.diff(np.append(cstart, len(cols)))
        groups = []
        i, n = 0, len(cu)
        while i < n:
            base = cu[i]
            pts = 0
            j = i
            while j < n and cu[j] < base + WIN and pts + ccount[j] <= GROUP_PT_CAP:
                pts += ccount[j]
                j += 1
            width = int(cu[j - 1] - base + 1)
            groups.append((int(base), i, j, int(pts), width))
            i = j
        per_core_groups.append((groups, cu, cstart, ccount, rows))

    # class counts -> per-class max across cores
    cls_counts = np.zeros((N_CORES, MAX_CLS + 1), np.int64)
    for c in range(N_CORES):
        for (base, i0, i1, pts, width) in per_core_groups[c][0]:
            cls_counts[c, (pts + P - 1) // P] += 1
    cls_max = cls_counts.max(axis=0)
    n_groups_tot = int(cls_max[1:].sum())
    # round groups to a multiple of G_BATCH (extra dummy groups, class 1)
    ng_round = ((n_groups_tot + G_BATCH - 1) // G_BATCH) * G_BATCH
    cls_max[1] += ng_round - n_groups_tot
    n_groups_tot = ng_round
    nch = int(sum(cls * cls_max[cls] for cls in range(1, MAX_CLS + 1)))
    nch = ((nch + J_TILE - 1) // J_TILE) * J_TILE

    structure = []                            # chunk-count per group, program order
    for cls in range(MAX_CLS, 0, -1):
        structure.extend([cls] * int(cls_max[cls]))

    per_core_meta = []
    for c in range(N_CORES):
        groups, cu, cstart, ccount, rows = per_core_groups[c]
        by_cls = {cls: [] for cls in range(1, MAX_CLS + 1)}
        for g in groups:
            by_cls[(g[3] + P - 1) // P].append(g)

        src = np.full(nch * P, -1, np.int64)      # source row in x (or -1 pad)
        lcell = np.full((P, nch), PAD_LCELL, np.float32)
        bases = np.zeros(n_groups_tot, np.int32)
        widths = np.zeros(n_groups_tot, np.int32)

        chunk_cursor = 0
        gslot = 0
        for cls in range(MAX_CLS, 0, -1):
            glist = by_cls[cls]
            for k in range(int(cls_max[cls])):
                if k < len(glist):
                    base, i0, i1, pts, width = glist[k]
                    bases[gslot] = base
                    widths[gslot] = width
                    s0 = cstart[i0]
                    seg_rows = rows[s0:s0 + pts]
                    seg_cols = (cu[np.repeat(np.arange(i0, i1), ccount[i0:i1])] - base)
                    o = chunk_cursor * P
                    src[o:o + pts] = seg_rows
                    ppos = np.arange(pts)
                    lcell[ppos % P, chunk_cursor + ppos // P] = seg_cols.astype(np.float32)
                else:
                    bases[gslot] = 0
                    widths[gslot] = 0
                gslot += 1
                chunk_cursor += cls
        # int16 scatter indices per group: [128] = base+t (t<width) else dump
        t = np.arange(WIN)
        sidx = np.where(t[None, :] < widths[:, None],
                        bases[:, None] + t[None, :],
                        DUMP_BASE + t[None, :]).astype(np.int16)  # [NG, 128]
        # wrap: ordinal i -> partition i%16, slot i//16; replicate across 8 Q7 groups
        NG = n_groups_tot
        wrapped = np.zeros((NG, 16, WIN // 16), np.int16)
        ii = np.arange(WIN)
        wrapped[:, ii % 16, ii // 16] = sidx[:, ii]
        # [NG, 128, W16] replicated across Q7 groups -> SBUF layout [128, NG*W16]
        sidx_w = np.ascontiguousarray(
            np.tile(wrapped, (1, 8, 1)).transpose(1, 0, 2)).reshape(P, -1)
        per_core_meta.append((src, lcell, bases, sidx_w))

    return per_core_meta, structure, nch, n_groups_tot


def _np_dtype():
    if DTYPE in ("bf16", "hilo"):
        import ml_dtypes
        return ml_dtypes.bfloat16
    return np.float32


def _row_width():
    return 2 * C if DTYPE == "hilo" else C


def _compact_rows(x_rows, src, nch):
    """Gather kept rows (cell-sorted, chunk-padded) into the DMA-tile layout.

    Device reads tiles of 4096 rows into SBUF [128, J*80]: compact row
    (t*4096 + p*J + j) must hold stream position (t*J + j)*128 + p.
    """
    npts = nch * P
    cw = _row_width()
    if DTYPE == "hilo":
        stream32 = np.zeros((npts, C), np.float32)
        valid = src >= 0
        stream32[valid] = x_rows[src[valid]]
        hi = stream32.astype(_np_dtype())
        lo = (stream32 - hi.astype(np.float32)).astype(_np_dtype())
        stream = np.concatenate([hi, lo], axis=1)          # [npts, 160]
    else:
        stream = np.zeros((npts, cw), _np_dtype())
        valid = src >= 0
        stream[valid] = x_rows[src[valid]]
    ntiles = nch // J_TILE
    return np.ascontiguousarray(
        stream.reshape(ntiles, J_TILE, P, cw).transpose(0, 2, 1, 3)
    ).reshape(npts, cw)


def _build_program(nch, n_groups_tot, structure):
    import concourse.mybir as mybir
    import concourse.tile as tile
    import concourse.bass as bass
    from concourse import bacc
    from concourse.bass import AP

    nc = bacc.Bacc(None)
    fdt = mybir.dt.bfloat16 if DTYPE in ("bf16", "hilo") else mybir.dt.float32
    cw = 2 * C if DTYPE == "hilo" else C
    xc_d = nc.declare_dram_parameter("xc", [nch * P, cw], fdt, isOutput=False)
    lcell_d = nc.declare_dram_parameter("lcell", [P, nch], fdt, isOutput=False)
    scatter_mode = FLUSH_MODE == "scatter"
    N_TABLES = 4
    if scatter_mode:
        sidx_d = nc.declare_dram_parameter(
            "sidx", [P, n_groups_tot * (WIN // 16)], mybir.dt.int16, isOutput=False)
        out_tabs = [nc.declare_dram_parameter(f"out{k}", [SPAN, P], mybir.dt.float32, isOutput=True)
                    for k in range(N_TABLES)]
    else:
        bases_d = nc.declare_dram_parameter("bases", [1, n_groups_tot], mybir.dt.int32, isOutput=False)
        out_d = nc.declare_dram_parameter("out", [C, SPAN], mybir.dt.float32, isOutput=True)

    with tile.TileContext(nc) as tc:
        with (
            tc.tile_pool(name="meta", bufs=1) as meta,
            tc.tile_pool(name="feats", bufs=4) as featsp,
            tc.tile_pool(name="oneh", bufs=4) as onehp,
            tc.tile_pool(name="psum", bufs=4, space="PSUM") as psump,
            tc.tile_pool(name="stage", bufs=3) as stagep,
            tc.tile_pool(name="const", bufs=1) as constp,
        ):
            lcell_sb = meta.tile([P, nch], fdt)
            nc.sync.dma_start(lcell_sb[:], lcell_d[:])
            if scatter_mode:
                sidx_sb = meta.tile([P, n_groups_tot * (WIN // 16)], mybir.dt.int16)
                nc.sync.dma_start(sidx_sb[:], sidx_d[:])
            else:
                bases_sb = meta.tile([1, n_groups_tot], mybir.dt.int32)
                nc.sync.dma_start(bases_sb[:], bases_d[:])
                accum = stagep.tile([C, SPAN], mybir.dt.float32, tag="acc")
                nc.vector.memset(accum[:], 0.0)

            iota_i = constp.tile([P, WIN], mybir.dt.int32)
            nc.gpsimd.iota(iota_i[:], pattern=[[1, WIN]], base=0, channel_multiplier=0)
            iota_f = constp.tile([P, WIN], fdt)
            nc.vector.tensor_copy(iota_f[:], iota_i[:])

            feats_tiles = [None] * (nch // J_TILE)
            oneh_tiles = [None] * (nch // J_TILE)

            def ensure_tile(t):
                if feats_tiles[t] is None:
                    ft = featsp.tile([P, J_TILE * cw], fdt)
                    nc.sync.dma_start(
                        ft[:], xc_d[t * J_TILE * P:(t + 1) * J_TILE * P, :])
                    feats_tiles[t] = ft
                    oh = onehp.tile([P, J_TILE * WIN], fdt)
                    if ONEHOT_BATCH3D:
                        lc = lcell_sb[:, t * J_TILE:(t + 1) * J_TILE]
                        in0 = AP(lc.tensor, lc.offset, list(lc.ap) + [[0, WIN]])
                        it = iota_f[:]
                        in1 = AP(it.tensor, it.offset,
                                 [list(it.ap[0]), [0, J_TILE], list(it.ap[1])])
                        out_oh = oh[:].rearrange("p (j w) -> p j w", w=WIN)
                        nc.vector.tensor_tensor(out=out_oh, in0=in0, in1=in1,
                                                op=mybir.AluOpType.is_equal)
                    else:
                        for s in range(J_TILE):
                            nc.vector.tensor_tensor(
                                out=oh[:, s * WIN:(s + 1) * WIN],
                                in0=lcell_sb[:, t * J_TILE + s:t * J_TILE + s + 1].to_broadcast([P, WIN]),
                                in1=iota_f[:],
                                op=mybir.AluOpType.is_equal)
                    oneh_tiles[t] = oh

            cursor = 0
            stage_tile = None
            for g, cls in enumerate(structure):
                if scatter_mode:
                    ps = psump.tile([WIN, C], mybir.dt.float32)
                else:
                    ps = psump.tile([C, WIN], mybir.dt.float32)
                for t in range(cls):
                    j = cursor + t
                    tid, slot = j // J_TILE, j % J_TILE
                    ensure_tile(tid)
                    oneh_ap = oneh_tiles[tid][:, slot * WIN:(slot + 1) * WIN]
                    if DTYPE == "hilo":
                        hi_ap = feats_tiles[tid][:, slot * cw:slot * cw + C]
                        lo_ap = feats_tiles[tid][:, slot * cw + C:(slot + 1) * cw]
                        nc.tensor.matmul(out=ps[:], lhsT=oneh_ap, rhs=hi_ap,
                                         start=(t == 0), stop=False)
                        nc.tensor.matmul(out=ps[:], lhsT=oneh_ap, rhs=lo_ap,
                                         start=False, stop=(t == cls - 1))
                        cursorless = None
                    else:
                        feats_ap = feats_tiles[tid][:, slot * C:(slot + 1) * C]
                        if scatter_mode:
                            nc.tensor.matmul(out=ps[:], lhsT=oneh_ap, rhs=feats_ap,
                                             start=(t == 0), stop=(t == cls - 1))
                        else:
                            nc.tensor.matmul(out=ps[:], lhsT=feats_ap, rhs=oneh_ap,
                                             start=(t == 0), stop=(t == cls - 1))
                cursor += cls
                if scatter_mode:
                    gb = g % G_BATCH
                    if gb == 0:
                        stage_tile = stagep.tile([P, G_BATCH * C], mybir.dt.float32)
                    nc.vector.tensor_copy(stage_tile[:, gb * C:(gb + 1) * C], ps[:])
                    if gb == G_BATCH - 1:
                        g0 = g - (G_BATCH - 1)
                        W16 = WIN // 16
                        tab = out_tabs[(g // G_BATCH) % N_TABLES]
                        nc.gpsimd.dma_scatter_add(
                            tab[:, :C],
                            stage_tile[:].rearrange("p (b e) -> p b e", e=C),
                            sidx_sb[:, g0 * W16:(g0 + G_BATCH) * W16],
                            G_BATCH * WIN, G_BATCH * WIN, C,
                            elem_step=P)
                else:
                    rv = nc.vector.value_load(bases_sb[0:1, g:g + 1],
                                              min_val=0, max_val=SPAN - WIN)
                    dst = accum[:, bass.ds(rv, WIN)]
                    nc.vector.tensor_tensor(out=dst, in0=dst, in1=ps[:],
                                            op=mybir.AluOpType.add)

            if not scatter_mode:
                nc.sync.dma_start(out_d[:], accum[:])
    nc.finalize()
    return nc


def kernel(x, camera2lidar, camera_intrinsics, img_aug_matrix, lidar_aug_matrix):
    global LAST_RESULTS
    from concourse.bass_utils import run_bass_kernel_spmd

    x = np.ascontiguousarray(np.asarray(x, dtype=np.float32))
    B, N, D, H, W, Cin = x.shape
    assert (B, N, D, H, W, Cin) == (1, 6, 118, 32, 88, 80), x.shape

    coords = _geometry_coords(camera2lidar, camera_intrinsics,
                              img_aug_matrix, lidar_aug_matrix)
    per_core_meta, structure, nch, n_groups_tot = _build_metadata(coords)

    x_rows = x.reshape(-1, C)
    if DTYPE == "bf16":
        x_rows = x_rows.astype(_np_dtype())
    scatter_mode = FLUSH_MODE == "scatter"
    in_maps = []
    for c in range(N_CORES):
        src, lcell, bases, sidx_w = per_core_meta[c]
        m = {"xc": _compact_rows(x_rows, src, nch),
             "lcell": lcell.astype(_np_dtype())}
        if scatter_mode:
            m["sidx"] = sidx_w
        else:
            m["bases"] = bases.reshape(1, -1)
        in_maps.append(m)

    nc = _build_program(nch, n_groups_tot, structure)
    res = run_bass_kernel_spmd(nc, in_maps, list(range(N_CORES)))
    LAST_RESULTS = res

    out = np.zeros((C, NXX * NXY), np.float32)
    for c in range(N_CORES):
        if scatter_mode:
            tab = sum(res.results[c][f"out{k}"].astype(np.float32) for k in range(4))
            shard = tab[:SHARD_COLS, :C].T
        else:
            shard = res.results[c]["out"][:, :SHARD_COLS]
        out[:, c::N_CORES] = shard
    return out.reshape(1, C, NXX, NXY)


# revision 14
# speedup vs baseline: 1.2645x; 1.1679x over previous
"""BEV view transform (lift-splat segment-sum) Trainium2 kernel.

Strategy
--------
Host: compute per-point BEV cell ids from the small calibration matrices
(exactly mirroring the reference jax ops), then shard points across the 8
NeuronCores by cell % 8 (output-interleaved, perfectly balanced) and build a
cell-sorted, chunk-padded compact copy of the kept feature rows per core.

Device (one SPMD program on 8 cores): stream the compact rows sequentially,
build one-hot matrices from per-point local-cell metadata (DVE is_equal vs an
iota row), accumulate each "group" (a 128-wide window of the core's cell
columns) in PSUM via chained matmuls (onehot^T @ feats -> [128 cells, 80]),
then flush each window into a per-core DRAM table [SPAN, 128] with
dma_scatter_add (row = local cell), tails redirected to a dump window.
Host reassembles [1, 80, 360, 360] from the 8 tables.

FLUSH_MODE:
  "scatter" (default): dma_scatter_add flush, no registers.
  "reg": SBUF accumulator + dynamic-offset DVE adds (smaller DMA traffic).
"""
import os
import numpy as np

# ---- problem constants (hardcoded; must match the reference task) ----
IMAGE_SIZE = (256, 704)
FEATURE_SIZE = (32, 88)
XBOUND = (-54.0, 54.0, 0.3)
YBOUND = (-54.0, 54.0, 0.3)
ZBOUND = (-10.0, 10.0, 20.0)
DBOUND = (1.0, 60.0, 0.5)
NXX = 360
NXY = 360
NXZ = 1
C = 80
P = 128

N_CORES = 8
SHARD_COLS = (NXX * NXY) // N_CORES          # 16200 local columns per core
WIN = 128                                    # group window (accumulator cols)
SPAN = 16384                                 # table rows / accumulator width
DUMP_BASE = SPAN - WIN                       # scatter dump window
GROUP_PT_CAP = 2048                          # max points per group (16 chunks)
MAX_CLS = GROUP_PT_CAP // P                  # 16
J_TILE = 32                                  # chunks per feats DMA tile
G_BATCH = 16                                 # groups per scatter_add call
PAD_LCELL = 20000.0                          # local-cell value for padding lanes
FLUSH_MODE = os.environ.get("BEV_FLUSH", "scatter")
ONEHOT_BATCH3D = os.environ.get("BEV_ONEHOT_3D", "1") == "1"
DTYPE = os.environ.get("BEV_DTYPE", "hilo")  # "f32" | "bf16" | "hilo"

LAST_RESULTS = None  # test harness can inspect exec_time_ns / profile


def _geometry_coords(camera2lidar, camera_intrinsics, img_aug_matrix, lidar_aug_matrix):
    """Per-point voxel coords, bit-identical to the reference jax computation.

    Runs on CPU jax: the reference graph itself only runs on CPU (neuron has
    no triangular-solve for linalg.inv), so the grading reference output is
    CPU-computed and we must match its float behavior exactly.
    """
    import jax
    with jax.default_device(jax.devices('cpu')[0]):
        return _geometry_coords_impl(camera2lidar, camera_intrinsics,
                                     img_aug_matrix, lidar_aug_matrix)


def _geometry_coords_impl(camera2lidar, camera_intrinsics, img_aug_matrix, lidar_aug_matrix):
    import jax.numpy as jnp
    from jax import lax

    DX = np.array([XBOUND[2], YBOUND[2], ZBOUND[2]], np.float32)
    BX = np.array([XBOUND[0] + XBOUND[2] / 2.0,
                   YBOUND[0] + YBOUND[2] / 2.0,
                   ZBOUND[0] + ZBOUND[2] / 2.0], np.float32)

    iH, iW = IMAGE_SIZE
    fH, fW = FEATURE_SIZE
    ds = jnp.arange(DBOUND[0], DBOUND[1], DBOUND[2], dtype=jnp.float32)
    xs = jnp.linspace(0.0, iW - 1.0, fW, dtype=jnp.float32)
    ys = jnp.linspace(0.0, iH - 1.0, fH, dtype=jnp.float32)
    d_g, y_g, x_g = jnp.meshgrid(ds, ys, xs, indexing='ij')
    frustum = jnp.stack([x_g, y_g, d_g], axis=-1)            # [D,fH,fW,3]

    camera2lidar = jnp.asarray(camera2lidar)
    camera_intrinsics = jnp.asarray(camera_intrinsics)
    img_aug_matrix = jnp.asarray(img_aug_matrix)
    lidar_aug_matrix = jnp.asarray(lidar_aug_matrix)

    intrins = camera_intrinsics[..., :3, :3]
    post_rots = img_aug_matrix[..., :3, :3]
    post_trans = img_aug_matrix[..., :3, 3]
    c2l_rots = camera2lidar[..., :3, :3]
    c2l_trans = camera2lidar[..., :3, 3]
    extra_rots = lidar_aug_matrix[..., :3, :3]
    extra_trans = lidar_aug_matrix[..., :3, 3]

    pts = frustum[None, None] - post_trans[:, :, None, None, None, :]
    pts = jnp.einsum('bnij,bndhwj->bndhwi', jnp.linalg.inv(post_rots), pts)
    pts = jnp.concatenate([pts[..., :2] * pts[..., 2:3], pts[..., 2:3]], axis=-1)
    combine = c2l_rots @ jnp.linalg.inv(intrins)
    pts = jnp.einsum('bnij,bndhwj->bndhwi', combine, pts) + c2l_trans[:, :, None, None, None, :]
    pts = jnp.einsum('bij,bndhwj->bndhwi', extra_rots, pts) + extra_trans[:, None, None, None, None, :]

    dx = jnp.asarray(DX)
    bx = jnp.asarray(BX)
    coords = ((lax.stop_gradient(pts) - (bx - dx / 2.0)) / dx).astype(jnp.int32)
    return np.asarray(coords).reshape(-1, 3)


def _build_metadata(coords):
    """Shard + sort + group.  Returns per-core metadata and structure info."""
    xi = coords[:, 0].astype(np.int64)
    yi = coords[:, 1].astype(np.int64)
    zi = coords[:, 2].astype(np.int64)
    kept = (xi >= 0) & (xi < NXX) & (yi >= 0) & (yi < NXY) & (zi >= 0) & (zi < NXZ)
    cell = xi * NXY + yi
    kept_rows = np.nonzero(kept)[0].astype(np.int64)
    cellk = cell[kept_rows]
    order = np.argsort(cellk, kind='stable')
    cellk = cellk[order]
    rowsk = kept_rows[order]

    cores = []
    for c in range(N_CORES):
        sel = (cellk % N_CORES) == c
        cores.append((cellk[sel] // N_CORES, rowsk[sel]))  # (local col sorted asc, src row)

    # group formation per core: window WIN local cols, <= GROUP_PT_CAP points,
    # whole cells only; class = exact number of chunks.
    per_core_groups = []
    for c in range(N_CORES):
        cols, rows = cores[c]
        cu, cstart = np.unique(cols, return_index=True)
        ccount = np.diff(np.append(cstart, len(cols)))
        groups = []
        i, n = 0, len(cu)
        while i < n:
            base = cu[i]
            pts = 0
            j = i
            while j < n and cu[j] < base + WIN and pts + ccount[j] <= GROUP_PT_CAP:
                pts += ccount[j]
                j += 1
            width = int(cu[j - 1] - base + 1)
            groups.append((int(base), i, j, int(pts), width))
            i = j
        per_core_groups.append((groups, cu, cstart, ccount, rows))

    # class counts -> per-class max across cores
    cls_counts = np.zeros((N_CORES, MAX_CLS + 1), np.int64)
    for c in range(N_CORES):
        for (base, i0, i1, pts, width) in per_core_groups[c][0]:
            cls_counts[c, (pts + P - 1) // P] += 1
    cls_max = cls_counts.max(axis=0)
    n_groups_tot = int(cls_max[1:].sum())
    # round groups to a multiple of G_BATCH (extra dummy groups, class 1)
    ng_round = ((n_groups_tot + G_BATCH - 1) // G_BATCH) * G_BATCH
    cls_max[1] += ng_round - n_groups_tot
    n_groups_tot = ng_round
    nch = int(sum(cls * cls_max[cls] for cls in range(1, MAX_CLS + 1)))
    nch = ((nch + J_TILE - 1) // J_TILE) * J_TILE

    structure = []                            # chunk-count per group, program order
    for cls in range(MAX_CLS, 0, -1):
        structure.extend([cls] * int(cls_max[cls]))

    per_core_meta = []
    for c in range(N_CORES):
        groups, cu, cstart, ccount, rows = per_core_groups[c]
        by_cls = {cls: [] for cls in range(1, MAX_CLS + 1)}
        for g in groups:
            by_cls[(g[3] + P - 1) // P].append(g)

        src = np.full(nch * P, -1, np.int64)      # source row in x (or -1 pad)
        lcell = np.full((P, nch), PAD_LCELL, np.float32)
        bases = np.zeros(n_groups_tot, np.int32)
        widths = np.zeros(n_groups_tot, np.int32)

        chunk_cursor = 0
        gslot = 0
        for cls in range(MAX_CLS, 0, -1):
            glist = by_cls[cls]
            for k in range(int(cls_max[cls])):
                if k < len(glist):
                    base, i0, i1, pts, width = glist[k]
                    bases[gslot] = base
                    widths[gslot] = width
                    s0 = cstart[i0]
                    seg_rows = rows[s0:s0 + pts]
                    seg_cols = (cu[np.repeat(np.arange(i0, i1), ccount[i0:i1])] - base)
                    o = chunk_cursor * P
                    src[o:o + pts] = seg_rows
                    ppos = np.arange(pts)
                    lcell[ppos % P, chunk_cursor + ppos // P] = seg_cols.astype(np.float32)
                else:
                    bases[gslot] = 0
                    widths[gslot] = 0
                gslot += 1
                chunk_cursor += cls
        # int16 scatter indices per group: [128] = base+t (t<width) else dump
        t = np.arange(WIN)
        sidx = np.where(t[None, :] < widths[:, None],
                        bases[:, None] + t[None, :],
                        DUMP_BASE + t[None, :]).astype(np.int16)  # [NG, 128]
        # wrap: ordinal i -> partition i%16, slot i//16; replicate across 8 Q7 groups
        NG = n_groups_tot
        wrapped = np.zeros((NG, 16, WIN // 16), np.int16)
        ii = np.arange(WIN)
        wrapped[:, ii % 16, ii // 16] = sidx[:, ii]
        # [NG, 128, W16] replicated across Q7 groups -> SBUF layout [128, NG*W16]
        sidx_w = np.ascontiguousarray(
            np.tile(wrapped, (1, 8, 1)).transpose(1, 0, 2)).reshape(P, -1)
        per_core_meta.append((src, lcell, bases, sidx_w))

    return per_core_meta, structure, nch, n_groups_tot


def _np_dtype():
    if DTYPE in ("bf16", "hilo"):
        import ml_dtypes
        return ml_dtypes.bfloat16
    return np.float32


def _row_width():
    return 2 * C if DTYPE == "hilo" else C


def _compact_rows(x_rows, src, nch):
    """Gather kept rows (cell-sorted, chunk-padded) into the DMA-tile layout.

    Device reads tiles of 4096 rows into SBUF [128, J*80]: compact row
    (t*4096 + p*J + j) must hold stream position (t*J + j)*128 + p.
    """
    npts = nch * P
    cw = _row_width()
    if DTYPE == "hilo":
        stream32 = np.zeros((npts, C), np.float32)
        valid = src >= 0
        stream32[valid] = x_rows[src[valid]]
        hi = stream32.astype(_np_dtype())
        lo = (stream32 - hi.astype(np.float32)).astype(_np_dtype())
        stream = np.concatenate([hi, lo], axis=1)          # [npts, 160]
    else:
        stream = np.zeros((npts, cw), _np_dtype())
        valid = src >= 0
        stream[valid] = x_rows[src[valid]]
    ntiles = nch // J_TILE
    return np.ascontiguousarray(
        stream.reshape(ntiles, J_TILE, P, cw).transpose(0, 2, 1, 3)
    ).reshape(npts, cw)


def _build_program(nch, n_groups_tot, structure):
    import concourse.mybir as mybir
    import concourse.tile as tile
    import concourse.bass as bass
    from concourse import bacc
    from concourse.bass import AP

    nc = bacc.Bacc(None)
    fdt = mybir.dt.bfloat16 if DTYPE in ("bf16", "hilo") else mybir.dt.float32
    cw = 2 * C if DTYPE == "hilo" else C
    xc_d = nc.declare_dram_parameter("xc", [nch * P, cw], fdt, isOutput=False)
    lcell_d = nc.declare_dram_parameter("lcell", [P, nch], fdt, isOutput=False)
    scatter_mode = FLUSH_MODE == "scatter"
    N_TABLES = 4
    if scatter_mode:
        sidx_d = nc.declare_dram_parameter(
            "sidx", [P, n_groups_tot * (WIN // 16)], mybir.dt.int16, isOutput=False)
        out_tabs = [nc.declare_dram_parameter(f"out{k}", [SPAN, P], mybir.dt.float32, isOutput=True)
                    for k in range(N_TABLES)]
    else:
        bases_d = nc.declare_dram_parameter("bases", [1, n_groups_tot], mybir.dt.int32, isOutput=False)
        out_d = nc.declare_dram_parameter("out", [C, SPAN], mybir.dt.float32, isOutput=True)

    with tile.TileContext(nc) as tc:
        with (
            tc.tile_pool(name="meta", bufs=1) as meta,
            tc.tile_pool(name="feats", bufs=4) as featsp,
            tc.tile_pool(name="oneh", bufs=4) as onehp,
            tc.tile_pool(name="psum", bufs=4, space="PSUM") as psump,
            tc.tile_pool(name="stage", bufs=3) as stagep,
            tc.tile_pool(name="const", bufs=1) as constp,
        ):
            lcell_sb = meta.tile([P, nch], fdt)
            nc.sync.dma_start(lcell_sb[:], lcell_d[:])
            if scatter_mode:
                sidx_sb = meta.tile([P, n_groups_tot * (WIN // 16)], mybir.dt.int16)
                nc.sync.dma_start(sidx_sb[:], sidx_d[:])
            else:
                bases_sb = meta.tile([1, n_groups_tot], mybir.dt.int32)
                nc.sync.dma_start(bases_sb[:], bases_d[:])
                accum = stagep.tile([C, SPAN], mybir.dt.float32, tag="acc")
                nc.vector.memset(accum[:], 0.0)

            iota_i = constp.tile([P, WIN], mybir.dt.int32)
            nc.gpsimd.iota(iota_i[:], pattern=[[1, WIN]], base=0, channel_multiplier=0)
            iota_f = constp.tile([P, WIN], fdt)
            nc.vector.tensor_copy(iota_f[:], iota_i[:])
            iota_rep = constp.tile([P, J_TILE * WIN], fdt)
            for s_ in range(J_TILE):
                nc.vector.tensor_copy(iota_rep[:, s_ * WIN:(s_ + 1) * WIN], iota_f[:])

            feats_tiles = [None] * (nch // J_TILE)
            oneh_tiles = [None] * (nch // J_TILE)

            def ensure_tile(t):
                if feats_tiles[t] is None:
                    ft = featsp.tile([P, J_TILE * cw], fdt)
                    nc.sync.dma_start(
                        ft[:], xc_d[t * J_TILE * P:(t + 1) * J_TILE * P, :])
                    feats_tiles[t] = ft
                    oh = onehp.tile([P, J_TILE * WIN], fdt)
                    if ONEHOT_BATCH3D:
                        lc = lcell_sb[:, t * J_TILE:(t + 1) * J_TILE]
                        in0 = AP(lc.tensor, lc.offset, list(lc.ap) + [[0, WIN]])
                        in1 = iota_rep[:].rearrange("p (j w) -> p j w", w=WIN)
                        out_oh = oh[:].rearrange("p (j w) -> p j w", w=WIN)
                        nc.vector.tensor_tensor(out=out_oh, in0=in0, in1=in1,
                                                op=mybir.AluOpType.is_equal)
                    else:
                        for s in range(J_TILE):
                            nc.vector.tensor_tensor(
                                out=oh[:, s * WIN:(s + 1) * WIN],
                                in0=lcell_sb[:, t * J_TILE + s:t * J_TILE + s + 1].to_broadcast([P, WIN]),
                                in1=iota_f[:],
                                op=mybir.AluOpType.is_equal)
                    oneh_tiles[t] = oh

            cursor = 0
            stage_tile = None
            for g, cls in enumerate(structure):
                if scatter_mode:
                    ps = psump.tile([WIN, C], mybir.dt.float32)
                else:
                    ps = psump.tile([C, WIN], mybir.dt.float32)
                for t in range(cls):
                    j = cursor + t
                    tid, slot = j // J_TILE, j % J_TILE
                    ensure_tile(tid)
                    oneh_ap = oneh_tiles[tid][:, slot * WIN:(slot + 1) * WIN]
                    if DTYPE == "hilo":
                        hi_ap = feats_tiles[tid][:, slot * cw:slot * cw + C]
                        lo_ap = feats_tiles[tid][:, slot * cw + C:(slot + 1) * cw]
                        nc.tensor.matmul(out=ps[:], lhsT=oneh_ap, rhs=hi_ap,
                                         start=(t == 0), stop=False)
                        nc.tensor.matmul(out=ps[:], lhsT=oneh_ap, rhs=lo_ap,
                                         start=False, stop=(t == cls - 1))
                        cursorless = None
                    else:
                        feats_ap = feats_tiles[tid][:, slot * C:(slot + 1) * C]
                        if scatter_mode:
                            nc.tensor.matmul(out=ps[:], lhsT=oneh_ap, rhs=feats_ap,
                                             start=(t == 0), stop=(t == cls - 1))
                        else:
                            nc.tensor.matmul(out=ps[:], lhsT=feats_ap, rhs=oneh_ap,
                                             start=(t == 0), stop=(t == cls - 1))
                cursor += cls
                if scatter_mode:
                    gb = g % G_BATCH
                    if gb == 0:
                        stage_tile = stagep.tile([P, G_BATCH * C], mybir.dt.float32)
                    nc.vector.tensor_copy(stage_tile[:, gb * C:(gb + 1) * C], ps[:])
                    if gb == G_BATCH - 1:
                        g0 = g - (G_BATCH - 1)
                        W16 = WIN // 16
                        tab = out_tabs[(g // G_BATCH) % N_TABLES]
                        nc.gpsimd.dma_scatter_add(
                            tab[:, :C],
                            stage_tile[:].rearrange("p (b e) -> p b e", e=C),
                            sidx_sb[:, g0 * W16:(g0 + G_BATCH) * W16],
                            G_BATCH * WIN, G_BATCH * WIN, C,
                            elem_step=P)
                else:
                    rv = nc.vector.value_load(bases_sb[0:1, g:g + 1],
                                              min_val=0, max_val=SPAN - WIN)
                    dst = accum[:, bass.ds(rv, WIN)]
                    nc.vector.tensor_tensor(out=dst, in0=dst, in1=ps[:],
                                            op=mybir.AluOpType.add)

            if not scatter_mode:
                nc.sync.dma_start(out_d[:], accum[:])
    nc.finalize()
    return nc


def kernel(x, camera2lidar, camera_intrinsics, img_aug_matrix, lidar_aug_matrix):
    global LAST_RESULTS
    from concourse.bass_utils import run_bass_kernel_spmd

    x = np.ascontiguousarray(np.asarray(x, dtype=np.float32))
    B, N, D, H, W, Cin = x.shape
    assert (B, N, D, H, W, Cin) == (1, 6, 118, 32, 88, 80), x.shape

    coords = _geometry_coords(camera2lidar, camera_intrinsics,
                              img_aug_matrix, lidar_aug_matrix)
    per_core_meta, structure, nch, n_groups_tot = _build_metadata(coords)

    x_rows = x.reshape(-1, C)
    if DTYPE == "bf16":
        x_rows = x_rows.astype(_np_dtype())
    scatter_mode = FLUSH_MODE == "scatter"
    in_maps = []
    for c in range(N_CORES):
        src, lcell, bases, sidx_w = per_core_meta[c]
        m = {"xc": _compact_rows(x_rows, src, nch),
             "lcell": lcell.astype(_np_dtype())}
        if scatter_mode:
            m["sidx"] = sidx_w
        else:
            m["bases"] = bases.reshape(1, -1)
        in_maps.append(m)

    nc = _build_program(nch, n_groups_tot, structure)
    res = run_bass_kernel_spmd(nc, in_maps, list(range(N_CORES)))
    LAST_RESULTS = res

    out = np.zeros((C, NXX * NXY), np.float32)
    for c in range(N_CORES):
        if scatter_mode:
            tab = sum(res.results[c][f"out{k}"].astype(np.float32) for k in range(4))
            shard = tab[:SHARD_COLS, :C].T
        else:
            shard = res.results[c]["out"][:, :SHARD_COLS]
        out[:, c::N_CORES] = shard
    return out.reshape(1, C, NXX, NXY)
